# revision 1
# baseline (speedup 1.0000x reference)
"""Trainium2 Bass kernel for causal multi-head attention with RoPE.

Problem: B=4, S=2048, D=768, H=12, HD=64 (torch-Linear style projections,
rotary embeddings on q/k, causal softmax, output projection + bias).

Sharding across 8 NeuronCores: core c handles batch c//2 and head-group
c%2 (6 of 12 heads). Each core computes a partial output projection
(its heads' contribution to ctx @ Wo.T); the host sums the two partials
per batch and adds the bias. No device collectives.

Per-core kernel (matmuls in fp32r/TF32):
  - Q^T/K^T [hd, S] computed directly via pre-transposed weights; RoPE
    fused into PSUM eviction (DVE muls against host cos/sin tables with
    rotate_half signs folded in; GpSimd does the final add). The
    projection is streamed in 512-column eT chunks, one head-pair per
    pass, interleaved with attention of already-finished pairs.
  - V [S, hd] with an appended ones column.
  - Per head: scores^T [k, q] = K_j^T.T @ Q^T (causal: q >= 128j only),
    exp on ScalarE with scale=1/8 folded in (no max subtraction; scores
    are bounded), PV accumulation with [V|1] stationary so row 64 of the
    PSUM accumulator is the softmax denominator for free. The ctx
    accumulator runs in two 1024-column halves so attention PSUM fits
    alongside the projection PSUM, and PV is emitted one k-chunk behind
    the scores/exp stream so PE's in-order queue never stalls on ScalarE.
  - Eviction writes unnormalized ctx^T plus reciprocal denominators;
    normalization happens off the critical path by DMA-broadcasting each
    reciprocal row across 64 partitions and multiplying in place, then
    out = ctx^T-chunks.T @ Wo^T-chunks.
"""

import numpy as np

B, S, D, H = 4, 2048, 768, 12
HD = D // H          # 64
N_CORES = 8
HEADS_PER_CORE = 6
PAIRS = 3            # head pairs per core
DC = D // 128        # 6 contraction chunks
MC = HEADS_PER_CORE * HD // 128   # 3 output-dim chunks (pairs)
NJ = S // 128        # 16 k-chunks
HW_ = 1024           # ctx half width

_CACHE = {}


def _rope_tables():
    inv_freq = 1.0 / (10000.0 ** (np.arange(0, HD, 2, dtype=np.float64) / HD))
    ang = np.arange(S, dtype=np.float64)[:, None] * inv_freq[None, :]  # [S, 32]
    cos = np.cos(ang).astype(np.float32)   # [S, 32]
    sin = np.sin(ang).astype(np.float32)
    cosF = np.empty((128, S), np.float32)
    sinM = np.empty((128, S), np.float32)
    for g in range(4):
        cosF[32 * g:32 * g + 32] = cos.T
        sgn = -1.0 if g % 2 == 0 else 1.0
        sinM[32 * g:32 * g + 32] = sgn * sin.T
    return cosF, sinM


def _build_program(reps=1):
    import concourse.bacc as bacc
    import concourse.mybir as mybir
    import concourse.tile as tile

    f32 = mybir.dt.float32
    f32r = mybir.dt.float32r
    AF = mybir.ActivationFunctionType
    OP = mybir.AluOpType

    nc = bacc.Bacc("TRN2", target_bir_lowering=False, debug=False,
                   num_devices=N_CORES)

    eT = nc.declare_dram_parameter("eT", [D, S], f32r, isOutput=False)
    wq = nc.declare_dram_parameter("wq", [D, 384], f32r, isOutput=False)
    wk = nc.declare_dram_parameter("wk", [D, 384], f32r, isOutput=False)
    wv = nc.declare_dram_parameter("wv", [D, 384], f32r, isOutput=False)
    wo = nc.declare_dram_parameter("wo", [384, D], f32r, isOutput=False)
    cosF_d = nc.declare_dram_parameter("cosF", [128, S], f32, isOutput=False)
    sinM_d = nc.declare_dram_parameter("sinM", [128, S], f32, isOutput=False)
    mask_d = nc.declare_dram_parameter("mask", [128, 128], f32, isOutput=False)
    o = nc.declare_dram_parameter("o", [S, D], f32, isOutput=True)

    with tile.TileContext(nc) as tc, \
            nc.allow_low_precision(reason="fp32r (tf32) matmul operand tiles"):
        with tc.tile_pool(name="const", bufs=1) as cp:
            cosF = cp.tile([128, S], f32)
            sinM = cp.tile([128, S], f32)
            msk = cp.tile([128, 128], f32)

            qt = cp.tile([128, PAIRS, S], f32r)
            kt = cp.tile([128, PAIRS, S], f32r)
            vt = cp.tile([128, NJ, HEADS_PER_CORE, HD + 1], f32r)
            nc.vector.memset(vt[:, :, :, HD].bitcast(mybir.dt.uint32),
                             0x3F800000)
            wot = cp.tile([128, MC, D], f32r)
            rcs = cp.tile([97, 3, HW_], f32)

            eT_r = eT[:].rearrange("(n p) s -> p n s", p=128)

            for _rep in range(reps):
                with (
                    # attention-side pools first (bottom of allocator stack)
                    tc.tile_pool(name="asb", bufs=3) as asb,
                    tc.tile_pool(name="pc", bufs=1) as pc,
                ):
                    cxt = pc.tile([128, PAIRS, S], f32r)   # unnormalized ctx^T
                    attn_psum = tc.tile_pool(name="scp", bufs=2, space="PSUM")
                    scp = attn_psum.__enter__()
                    cxp_cm = tc.tile_pool(name="cxp", bufs=1, space="PSUM")
                    cxp = cxp_cm.__enter__()
                    pps_cm = tc.tile_pool(name="pps", bufs=2, space="PSUM")
                    pps = pps_cm.__enter__()

                    def qk_chunk(pairs, cc, pjs, wqt, wkt, wvt):
                        """One 512-col chunk of Q^T/K^T (+V when pair 0)."""
                        cols = slice(512 * cc, 512 * cc + 512)
                        etA = pjs.tile([128, 3, 512], f32r, tag="et",
                                       bufs=3, name=f"eA{pairs[0]}{cc}")
                        nc.sync.dma_start(etA[:], eT_r[:, 0:3, cols])
                        etB = pjs.tile([128, 3, 512], f32r, tag="et",
                                       bufs=3, name=f"eB{pairs[0]}{cc}")
                        nc.sync.dma_start(etB[:], eT_r[:, 3:6, cols])

                        def et(d):
                            return (etA if d < 3 else etB)[:, d % 3, :]

                        for pair in pairs:
                            for wt, dst in ((wqt, qt), (wkt, kt)):
                                ps = pps.tile([128, 512], f32, tag="ps",
                                              name=f"ps{pair}{cc}")
                                for d in range(DC):
                                    nc.tensor.matmul(
                                        ps[:],
                                        wt[:, d, 128 * pair:128 * pair + 128],
                                        et(d),
                                        start=(d == 0), stop=(d == DC - 1))
                                t_t = pjs.tile([128, 512], f32, tag="t",
                                               bufs=2, name=f"t{pair}{cc}")
                                nc.vector.tensor_tensor(
                                    t_t[:], ps[:], cosF[:, cols], OP.mult)
                                u_t = pjs.tile([128, 512], f32, tag="u",
                                               bufs=2, name=f"u{pair}{cc}")
                                for g in range(4):
                                    gd = slice(32 * g, 32 * g + 32)
                                    gs = slice(32 * (g ^ 1), 32 * (g ^ 1) + 32)
                                    nc.vector.tensor_tensor(
                                        u_t[gd], ps[gs], sinM[gd, cols],
                                        OP.mult)
                                nc.gpsimd.tensor_tensor(
                                    dst[:, pair, cols], t_t[:], u_t[:], OP.add)
                        if 0 in pairs:
                            for i in range(4 * cc, 4 * cc + 4):
                                io = 128 * (i % 4)
                                pv = pps.tile([128, 384], f32, tag="ps",
                                              name=f"pv{cc}{i}")
                                for d in range(DC):
                                    nc.tensor.matmul(
                                        pv[:],
                                        et(d)[:, io:io + 128],
                                        wvt[:, d, :],
                                        start=(d == 0), stop=(d == DC - 1))
                                nc.vector.tensor_copy(vt[:, i, :, 0:HD], pv[:])

                    def norm_head(h):
                        pair, po = h // 2, 64 * (h % 2)
                        for half in range(2):
                            idx = 2 * h + half
                            cs = slice(HW_ * half, HW_ * half + HW_)
                            rp, rf = 32 * (idx % 4), idx // 4
                            rbs = asb.tile([128, HW_], f32, tag="rbs", bufs=2,
                                           name=f"rbs{h}{half}")
                            nc.sync.dma_start(
                                rbs[po:po + HD, :],
                                rcs[rp:rp + 1, rf, None, :]
                                .to_broadcast([1, HD, HW_]))
                            nc.vector.tensor_tensor(
                                cxt[po:po + HD, pair, cs],
                                cxt[po:po + HD, pair, cs],
                                rbs[po:po + HD, :], OP.mult)

                    def attn_half(h, half, cpool=None):
                        pair, po = h // 2, 64 * (h % 2)
                        qb = HW_ * half
                        C = (cpool or cxp).tile([HD + 1, HW_], f32, tag="C",
                                                name=f"C{h}{half}")
                        nj = 8 if half == 0 else NJ

                        def emit_pv(j, et_, qlo):
                            c0 = qlo
                            while c0 < qb + HW_:
                                c1 = min((c0 // 512 + 1) * 512, qb + HW_)
                                nc.tensor.matmul(
                                    C[:, c0 - qb:c1 - qb],
                                    vt[:, j, h, :],
                                    et_[:, c0 - qlo:c1 - qlo],
                                    start=(j == 0),
                                    stop=(j == 4 * (c0 // 512) + 3))
                                c0 = c1

                        pending = None
                        for j in range(nj):
                            qlo = max(qb, 128 * j)
                            w = qb + HW_ - qlo
                            kk = slice(128 * j, 128 * j + 128)
                            sc = scp.tile([128, HW_], f32, tag="sc",
                                          name=f"sc{h}{half}{j}")
                            for c0 in range(0, w, 512):
                                cw = min(512, w - c0)
                                nc.tensor.matmul(
                                    sc[:, c0:c0 + cw],
                                    kt[po:po + HD, pair, kk],
                                    qt[po:po + HD, pair,
                                       qlo + c0:qlo + c0 + cw],
                                    start=True, stop=True)
                            et_ = asb.tile([128, HW_], f32r, tag="et_", bufs=3,
                                           name=f"ex{h}{half}{j}")
                            nc.scalar.activation(
                                et_[:, 0:w], sc[:, 0:w], AF.Exp,
                                scale=0.125)
                            if qlo == 128 * j:   # diagonal: zero k > q
                                nc.gpsimd.tensor_tensor(
                                    et_[:, 0:128], et_[:, 0:128], msk[:],
                                    OP.mult)
                            # software pipeline: PV runs one j behind so
                            # PE's in-order queue never waits on exp_j
                            if pending is not None:
                                emit_pv(*pending)
                            pending = (j, et_, qlo)
                        emit_pv(*pending)
                        cs = slice(qb, qb + HW_)
                        idx = 2 * h + half
                        rp, rf = 32 * (idx % 4), idx // 4
                        nc.vector.tensor_copy(cxt[po:po + HD, pair, cs],
                                              C[0:HD, :])
                        nc.vector.reciprocal(rcs[rp:rp + 1, rf, :],
                                             C[HD:HD + 1, :])

                    with tc.tile_pool(name="projsb", bufs=1) as pjs:
                        # weights/tables go on the ScalarE DMA queue so the
                        # eT stream (SP queue) starts immediately
                        wqt = pjs.tile([128, DC, 384], f32r)
                        nc.scalar.dma_start(
                            wqt[:], wq[:].rearrange("(n p) m -> p n m", p=128))
                        wkt = pjs.tile([128, DC, 384], f32r)
                        nc.scalar.dma_start(
                            wkt[:], wk[:].rearrange("(n p) m -> p n m", p=128))
                        nc.scalar.dma_start(cosF[:], cosF_d[:])
                        nc.scalar.dma_start(sinM[:], sinM_d[:])
                        nc.scalar.dma_start(msk[:], mask_d[:])
                        with tc.tile_pool(name="wvp", bufs=1) as wvp:
                            wvt = wvp.tile([128, DC, 384], f32r)
                            nc.scalar.dma_start(
                                wvt[:],
                                wv[:].rearrange("(n p) m -> p n m", p=128))
                            qk_chunk([0], 0, pjs, wqt, wkt, wvt)
                            qk_chunk([0], 1, pjs, wqt, wkt, wvt)
                            attn_half(0, 0)
                            qk_chunk([0], 2, pjs, wqt, wkt, wvt)
                            attn_half(1, 0)
                            qk_chunk([0], 3, pjs, wqt, wkt, wvt)
                            attn_half(0, 1)
                        nc.sync.dma_start(
                            wot[:], wo[:].rearrange("(n p) m -> p n m", p=128))
                        attn_half(1, 1)
                        qk_chunk([1, 2], 0, pjs, wqt, wkt, None)
                        qk_chunk([1, 2], 1, pjs, wqt, wkt, None)
                        attn_half(2, 0)
                        qk_chunk([1, 2], 2, pjs, wqt, wkt, None)
                        attn_half(3, 0)
                        qk_chunk([1, 2], 3, pjs, wqt, wkt, None)
                        # projection PSUM is dead now — recycle its banks as
                        # a second ctx accumulator so tail halves double-buffer
                        pps_cm.__exit__(None, None, None)
                        cxp2_cm = tc.tile_pool(name="cxp2", bufs=1,
                                               space="PSUM")
                        cxp2 = cxp2_cm.__enter__()
                        attn_half(2, 1)
                        norm_head(0)
                        attn_half(3, 1, cxp2)
                        norm_head(1)
                        attn_half(4, 0)
                        attn_half(5, 0, cxp2)
                        norm_head(2)
                        attn_half(4, 1)
                        norm_head(3)
                        attn_half(5, 1, cxp2)
                        norm_head(4)
                        norm_head(5)

                    cxp2_cm.__exit__(None, None, None)
                    cxp_cm.__exit__(None, None, None)
                    attn_psum.__exit__(None, None, None)

                    # ---- normalize + output projection ----
                    with (
                        tc.tile_pool(name="osb", bufs=1) as osb,
                        tc.tile_pool(name="osp", bufs=2, space="PSUM") as osp,
                    ):
                        for i in range(NJ):
                            op_ = osp.tile([128, D], f32, tag="op",
                                           name=f"op{i}")
                            ss = slice(128 * i, 128 * i + 128)
                            for pair in range(PAIRS):
                                for c0 in range(0, D, 512):
                                    cw = min(512, D - c0)
                                    nc.tensor.matmul(
                                        op_[:, c0:c0 + cw],
                                        cxt[:, pair, ss],
                                        wot[:, pair, c0:c0 + cw],
                                        start=(pair == 0),
                                        stop=(pair == PAIRS - 1))
                            ot = osb.tile([128, D], f32, tag="ot", bufs=3,
                                          name=f"ot{i}")
                            nc.scalar.copy(ot[:], op_[:])
                            eng = nc.sync if i % 2 == 0 else nc.scalar
                            eng.dma_start(o[ss, :], ot[:])

    nc.compile()
    return nc


def _get_program(reps=1):
    if reps not in _CACHE:
        _CACHE[reps] = _build_program(reps)
    return _CACHE[reps]


def _tf32_round(x):
    """Round-to-nearest-even to TF32 (10-bit mantissa), kept as float32 bits."""
    b = np.ascontiguousarray(x, np.float32).view(np.uint32)
    lsb = (b >> 13) & 1
    b = (b + np.uint32(0x0FFF) + lsb) & np.uint32(0xFFFFE000)
    return b.view(np.float32)


def make_in_maps(embeds, Wq, Wk, Wv, Wo):
    cosF, sinM = _rope_tables()
    mask = (np.arange(128)[:, None] <= np.arange(128)[None, :]).astype(np.float32)
    eTs = [_tf32_round(embeds[b].T) for b in range(B)]
    in_maps = []
    for c in range(N_CORES):
        b, hg = c // 2, c % 2
        hs = slice(hg * 384, hg * 384 + 384)
        in_maps.append({
            "eT": eTs[b],
            "wq": _tf32_round(Wq[hs].T),
            "wk": _tf32_round(Wk[hs].T),
            "wv": _tf32_round(Wv[hs].T),
            "wo": _tf32_round(Wo[:, hs].T),
            "cosF": cosF, "sinM": sinM, "mask": mask,
        })
    return in_maps


def kernel(embeds, Wq, Wk, Wv, Wo, bo):
    from concourse.bass_utils import run_bass_kernel_spmd

    embeds = np.asarray(embeds, np.float32)
    Wq = np.asarray(Wq, np.float32)
    Wk = np.asarray(Wk, np.float32)
    Wv = np.asarray(Wv, np.float32)
    Wo = np.asarray(Wo, np.float32)
    bo = np.asarray(bo, np.float32)

    nc = _get_program()
    in_maps = make_in_maps(embeds, Wq, Wk, Wv, Wo)
    res = run_bass_kernel_spmd(nc, in_maps, list(range(N_CORES))).results
    out = np.empty((B, S, D), np.float32)
    for b in range(B):
        out[b] = res[2 * b]["o"] + res[2 * b + 1]["o"] + bo
    return out



# revision 13
# speedup vs baseline: 1.2825x; 1.2825x over previous
"""Trainium2 Bass kernel for causal multi-head attention with RoPE.

Problem: B=4, S=2048, D=768, H=12, HD=64 (torch-Linear style projections,
rotary embeddings on q/k, causal softmax, output projection + bias).

Sharding across 8 NeuronCores: core c handles batch c//2 and head-group
c%2 (6 of 12 heads). Each core computes a partial output projection
(its heads' contribution to ctx @ Wo.T); the host sums the two partials
per batch and adds the bias. No device collectives.

Per-core kernel (all matmul operands bf16, fp32 PSUM accumulation):
  - Q^T/K^T [hd, S] via pre-transposed weights. RoPE: rotate_half is a
    partition permutation done by 4 small PSUM->SBUF DMAs; then two
    full-partition DVE multiplies (cos, sign-folded sin) and a GpSimd add.
  - V [S, hd] per head with an appended ones column (row 64 of the PV
    accumulator becomes the softmax denominator for free).
  - Attention in 512-query windows, per head-pair: scores for heads A and
    B issue back-to-back with stationaries in PE row groups 0-63/64-127,
    so the two matmuls run concurrently (row tiling). One exp covers both
    heads' PSUM banks via a [128, 2, w] access pattern (scale=1/8 folded
    in, no max subtraction; scores are bounded). PV runs one k-chunk
    behind the exp stream so PE's in-order queue never stalls on ScalarE.
  - Denominator row is DMA-broadcast to 64 partitions, reciprocal'd with
    64 active lanes, and multiplied into the evicted ctx tile.
  - out = ctx^T-chunks.T @ Wo^T-chunks, interleaved with the tail of the
    attention so PE never idles; partial output summed on host.
"""

import numpy as np

B, S, D, H = 4, 2048, 768, 12
HD = D // H          # 64
N_CORES = 8
HEADS_PER_CORE = 6
PAIRS = 3            # head pairs per core
DC = D // 128        # 6 contraction chunks
NJ = S // 128        # 16 k-chunks
W = 512              # q-window width
NW = S // W          # 4 windows

_CACHE = {}


def _rope_tables():
    inv_freq = 1.0 / (10000.0 ** (np.arange(0, HD, 2, dtype=np.float64) / HD))
    ang = np.arange(S, dtype=np.float64)[:, None] * inv_freq[None, :]  # [S, 32]
    cos = np.cos(ang).astype(np.float32)   # [S, 32]
    sin = np.sin(ang).astype(np.float32)
    cosF = np.empty((128, S), np.float32)
    sinM = np.empty((128, S), np.float32)
    for g in range(4):
        cosF[32 * g:32 * g + 32] = cos.T
        sgn = -1.0 if g % 2 == 0 else 1.0
        sinM[32 * g:32 * g + 32] = sgn * sin.T
    return cosF, sinM


def _build_program(reps=1, dbg=False):
    import concourse.bacc as bacc
    import concourse.mybir as mybir
    import concourse.tile as tile

    f32 = mybir.dt.float32
    bf16 = mybir.dt.bfloat16
    AF = mybir.ActivationFunctionType
    OP = mybir.AluOpType

    nc = bacc.Bacc("TRN2", target_bir_lowering=False, debug=False,
                   num_devices=N_CORES)

    eT = nc.declare_dram_parameter("eT", [D, S], bf16, isOutput=False)
    wq = nc.declare_dram_parameter("wq", [D, 384], bf16, isOutput=False)
    wk = nc.declare_dram_parameter("wk", [D, 384], bf16, isOutput=False)
    wv = nc.declare_dram_parameter("wv", [D, 384], bf16, isOutput=False)
    wo = nc.declare_dram_parameter("wo", [384, D], bf16, isOutput=False)
    cosF_d = nc.declare_dram_parameter("cosF", [128, S], bf16, isOutput=False)
    sinM_d = nc.declare_dram_parameter("sinM", [128, S], bf16, isOutput=False)
    mask_d = nc.declare_dram_parameter("mask", [128, 128], bf16, isOutput=False)
    o = nc.declare_dram_parameter("o", [S, D], f32, isOutput=True)
    if dbg:
        qtd = nc.declare_dram_parameter("qtd", [128, PAIRS, S], bf16,
                                        isOutput=True)
        ktd = nc.declare_dram_parameter("ktd", [128, PAIRS, S], bf16,
                                        isOutput=True)
        vtd = nc.declare_dram_parameter("vtd", [128, NJ, HEADS_PER_CORE,
                                                HD + 1], bf16, isOutput=True)
        cxtd = nc.declare_dram_parameter("cxtd", [128, PAIRS, S], bf16,
                                         isOutput=True)

    with tile.TileContext(nc) as tc, \
            nc.allow_low_precision(reason="bf16 matmul operand tiles"):
        with tc.tile_pool(name="const", bufs=1) as cp:
            cosF = cp.tile([128, S], bf16)
            sinM = cp.tile([128, S], bf16)
            msk = cp.tile([128, 128], bf16)

            qt = cp.tile([128, PAIRS, S], bf16)
            kt = cp.tile([128, PAIRS, S], bf16)
            vt = cp.tile([128, NJ, HEADS_PER_CORE, HD + 1], bf16)
            nc.vector.memset(vt[:, :, :, HD], 1.0)
            cxt = cp.tile([128, PAIRS, S], bf16)
            wot = cp.tile([128, PAIRS, D], bf16)

            eT_r = eT[:].rearrange("(n p) s -> p n s", p=128)

            for _rep in range(reps):
                with (
                    tc.tile_pool(name="asb", bufs=3) as asb,
                    tc.tile_pool(name="projsb", bufs=1) as pjs,
                ):
                    scp_cm = tc.tile_pool(name="scp", bufs=2, space="PSUM")
                    scp = scp_cm.__enter__()
                    cxp_cm = tc.tile_pool(name="cxp", bufs=2, space="PSUM")
                    cxp = cxp_cm.__enter__()
                    pps_cm = tc.tile_pool(name="pps", bufs=2, space="PSUM")
                    pps = pps_cm.__enter__()

                    def proj_chunk(pair, cc, with_v, wqt, wkt, wvt):
                        """One 512-col chunk of Q^T/K^T (+V when with_v)."""
                        cols = slice(512 * cc, 512 * cc + 512)
                        etA = pjs.tile([128, 3, 512], bf16, tag="et",
                                       bufs=3, name=f"eA{pair}{cc}")
                        nc.sync.dma_start(etA[:], eT_r[:, 0:3, cols])
                        etB = pjs.tile([128, 3, 512], bf16, tag="et",
                                       bufs=3, name=f"eB{pair}{cc}")
                        nc.sync.dma_start(etB[:], eT_r[:, 3:6, cols])

                        def et(d):
                            return (etA if d < 3 else etB)[:, d % 3, :]

                        for wt, dst in ((wqt, qt), (wkt, kt)):
                            ps = pps.tile([128, 512], f32, tag="ps",
                                          name=f"ps{pair}{cc}")
                            for d in range(DC):
                                nc.tensor.matmul(
                                    ps[:],
                                    wt[:, d, 128 * pair:128 * pair + 128],
                                    et(d),
                                    start=(d == 0), stop=(d == DC - 1))
                            # evict to bf16, then rotate_half as a
                            # partition-permuted SBUF->SBUF DMA
                            cpe = pjs.tile([128, 512], bf16, tag="cpe",
                                           bufs=2, name=f"e{pair}{cc}")
                            nc.vector.tensor_copy(cpe[:], ps[:])
                            cpr = pjs.tile([128, 512], bf16, tag="cpr",
                                           bufs=2, name=f"r{pair}{cc}")
                            for g in range(4):
                                gd = slice(32 * g, 32 * g + 32)
                                gs = slice(32 * (g ^ 1), 32 * (g ^ 1) + 32)
                                nc.scalar.dma_start(cpr[gd, :], cpe[gs, :])
                            t_t = pjs.tile([128, 512], bf16, tag="t",
                                           bufs=2, name=f"t{pair}{cc}")
                            nc.vector.tensor_tensor(
                                t_t[:], cpe[:], cosF[:, cols], OP.mult)
                            u_t = pjs.tile([128, 512], bf16, tag="u",
                                           bufs=2, name=f"u{pair}{cc}")
                            nc.vector.tensor_tensor(
                                u_t[:], cpr[:], sinM[:, cols], OP.mult)
                            nc.gpsimd.tensor_tensor(
                                dst[:, pair, cols], t_t[:], u_t[:], OP.add)
                        if with_v:
                            for i in range(4 * cc, 4 * cc + 4):
                                io = 128 * (i % 4)
                                pv = pps.tile([128, 384], f32, tag="ps",
                                              name=f"pv{cc}{i}")
                                for d in range(DC):
                                    nc.tensor.matmul(
                                        pv[:],
                                        et(d)[:, io:io + 128],
                                        wvt[:, d, :],
                                        start=(d == 0), stop=(d == DC - 1))
                                nc.vector.tensor_copy(vt[:, i, :, 0:HD], pv[:])

                    def attn_win(pair, w0):
                        """Attention for both heads of `pair` on q-window w0."""
                        base = W * w0
                        nj = 4 * w0 + 4
                        CA = cxp.tile([HD + 1, W], f32, tag="C",
                                      name=f"CA{pair}{w0}")
                        CB = cxp.tile([HD + 1, W], f32, tag="C",
                                      name=f"CB{pair}{w0}")

                        def emit_pv(j, et_, qlo):
                            off = qlo - base
                            wj = W - off
                            for hx, C in ((0, CA), (1, CB)):
                                nc.tensor.matmul(
                                    C[:, off:W],
                                    vt[:, j, 2 * pair + hx, :],
                                    et_[:, hx, 0:wj],
                                    start=(j == 0), stop=(j == nj - 1))

                        pend = None
                        for j in range(nj):
                            qlo = max(base, 128 * j)
                            wj = base + W - qlo
                            kk = slice(128 * j, 128 * j + 128)
                            sc = scp.tile([128, 2, W], f32, tag="sc",
                                          name=f"sc{pair}{w0}{j}")
                            nc.tensor.matmul(
                                sc[:, 0, 0:wj],
                                kt[0:HD, pair, kk],
                                qt[0:HD, pair, qlo:qlo + wj],
                                start=True, stop=True)
                            nc.tensor.matmul(
                                sc[:, 1, 0:wj],
                                kt[HD:128, pair, kk],
                                qt[HD:128, pair, qlo:qlo + wj],
                                start=True, stop=True)
                            et_ = asb.tile([128, 2, W], bf16, tag="ex",
                                           bufs=3, name=f"ex{pair}{w0}{j}")
                            nc.scalar.activation(
                                et_[:, :, 0:wj], sc[:, :, 0:wj], AF.Exp,
                                scale=0.125)
                            if qlo == 128 * j:   # diagonal: zero k > q
                                nc.gpsimd.tensor_tensor(
                                    et_[:, 0, 0:128], et_[:, 0, 0:128],
                                    msk[:], OP.mult)
                                nc.gpsimd.tensor_tensor(
                                    et_[:, 1, 0:128], et_[:, 1, 0:128],
                                    msk[:], OP.mult)
                            # software pipeline: PV runs one j behind so
                            # PE's in-order queue never waits on exp_j
                            if pend is not None:
                                emit_pv(*pend)
                            pend = (j, et_, qlo)
                        emit_pv(*pend)

                        cs = slice(base, base + W)
                        rec = asb.tile([128, W], bf16, tag="rec", bufs=2,
                                       name=f"rc{pair}{w0}")
                        for hx, C in ((0, CA), (1, CB)):
                            po = HD * hx
                            nc.vector.tensor_copy(cxt[po:po + HD, pair, cs],
                                                  C[0:HD, :])
                            rr = asb.tile([1, W], bf16, tag="rr", bufs=2,
                                          name=f"rr{pair}{w0}{hx}")
                            nc.vector.reciprocal(rr[:], C[HD:HD + 1, :])
                            nc.sync.dma_start(
                                rec[po:po + HD, :],
                                rr[0:1, None, :].to_broadcast([1, HD, W]))
                            nc.vector.tensor_tensor(
                                cxt[po:po + HD, pair, cs],
                                cxt[po:po + HD, pair, cs],
                                rec[po:po + HD, :], OP.mult)

                    def out_chunk(i, osp):
                        op_ = osp.tile([128, D], f32, tag="op", name=f"op{i}")
                        ss = slice(128 * i, 128 * i + 128)
                        for pair in range(PAIRS):
                            for c0 in range(0, D, 512):
                                cw = min(512, D - c0)
                                nc.tensor.matmul(
                                    op_[:, c0:c0 + cw],
                                    cxt[:, pair, ss],
                                    wot[:, pair, c0:c0 + cw],
                                    start=(pair == 0),
                                    stop=(pair == PAIRS - 1))
                        ot = asb.tile([128, D], f32, tag="ot", bufs=3,
                                      name=f"ot{i}")
                        nc.vector.tensor_copy(ot[:], op_[:])
                        eng = nc.sync if i % 2 == 0 else nc.scalar
                        eng.dma_start(o[ss, :], ot[:])

                    # weights/tables on the ScalarE DMA queue so the eT
                    # stream (SP queue) starts immediately
                    wqt = pjs.tile([128, DC, 384], bf16)
                    nc.scalar.dma_start(
                        wqt[:], wq[:].rearrange("(n p) m -> p n m", p=128))
                    wkt = pjs.tile([128, DC, 384], bf16)
                    nc.scalar.dma_start(
                        wkt[:], wk[:].rearrange("(n p) m -> p n m", p=128))
                    wvt = pjs.tile([128, DC, 384], bf16)
                    nc.scalar.dma_start(
                        wvt[:], wv[:].rearrange("(n p) m -> p n m", p=128))
                    nc.scalar.dma_start(cosF[:], cosF_d[:])
                    nc.scalar.dma_start(sinM[:], sinM_d[:])
                    nc.scalar.dma_start(msk[:], mask_d[:])
                    nc.sync.dma_start(
                        wot[:], wo[:].rearrange("(n p) m -> p n m", p=128))

                    # pipeline: V + pair p projections feed attention windows;
                    # next pair's projections overlap current pair's attention
                    proj_chunk(0, 0, True, wqt, wkt, wvt)
                    proj_chunk(0, 1, True, wqt, wkt, wvt)
                    attn_win(0, 0)
                    proj_chunk(0, 2, True, wqt, wkt, wvt)
                    attn_win(0, 1)
                    proj_chunk(0, 3, True, wqt, wkt, wvt)
                    attn_win(0, 2)
                    proj_chunk(1, 0, False, wqt, wkt, wvt)
                    attn_win(0, 3)
                    proj_chunk(1, 1, False, wqt, wkt, wvt)
                    attn_win(1, 0)
                    proj_chunk(1, 2, False, wqt, wkt, wvt)
                    attn_win(1, 1)
                    proj_chunk(1, 3, False, wqt, wkt, wvt)
                    attn_win(1, 2)
                    proj_chunk(2, 0, False, wqt, wkt, wvt)
                    attn_win(1, 3)
                    proj_chunk(2, 1, False, wqt, wkt, wvt)
                    proj_chunk(2, 2, False, wqt, wkt, wvt)
                    proj_chunk(2, 3, False, wqt, wkt, wvt)
                    # projection PSUM banks are dead now — recycle for the
                    # output projection so it overlaps pair-2 attention
                    pps_cm.__exit__(None, None, None)
                    osp_cm = tc.tile_pool(name="osp", bufs=1, space="PSUM")
                    osp = osp_cm.__enter__()
                    attn_win(2, 0)
                    attn_win(2, 1)
                    for i in range(0, 4):
                        out_chunk(i, osp)
                    attn_win(2, 2)
                    for i in range(4, 8):
                        out_chunk(i, osp)
                    attn_win(2, 3)
                    for i in range(8, 12):
                        out_chunk(i, osp)
                    osp_cm.__exit__(None, None, None)
                    cxp_cm.__exit__(None, None, None)
                    scp_cm.__exit__(None, None, None)
                    osp2_cm = tc.tile_pool(name="osp2", bufs=2, space="PSUM")
                    osp2 = osp2_cm.__enter__()
                    for i in range(12, 16):
                        out_chunk(i, osp2)
                    osp2_cm.__exit__(None, None, None)
                    if dbg:
                        nc.sync.dma_start(qtd[:], qt[:])
                        nc.sync.dma_start(ktd[:], kt[:])
                        nc.sync.dma_start(vtd[:], vt[:])
                        nc.sync.dma_start(cxtd[:], cxt[:])

    nc.compile()
    return nc


def _get_program(reps=1):
    if reps not in _CACHE:
        _CACHE[reps] = _build_program(reps)
    return _CACHE[reps]


def make_in_maps(embeds, Wq, Wk, Wv, Wo):
    import ml_dtypes
    bf16 = ml_dtypes.bfloat16
    cosF, sinM = _rope_tables()
    cosF, sinM = cosF.astype(bf16), sinM.astype(bf16)
    mask = (np.arange(128)[:, None] <= np.arange(128)[None, :]).astype(bf16)
    eTs = [np.ascontiguousarray(embeds[b].T).astype(bf16) for b in range(B)]
    in_maps = []
    for c in range(N_CORES):
        b, hg = c // 2, c % 2
        hs = slice(hg * 384, hg * 384 + 384)
        in_maps.append({
            "eT": eTs[b],
            "wq": np.ascontiguousarray(Wq[hs].T).astype(bf16),
            "wk": np.ascontiguousarray(Wk[hs].T).astype(bf16),
            "wv": np.ascontiguousarray(Wv[hs].T).astype(bf16),
            "wo": np.ascontiguousarray(Wo[:, hs].T).astype(bf16),
            "cosF": cosF, "sinM": sinM, "mask": mask,
        })
    return in_maps


def kernel(embeds, Wq, Wk, Wv, Wo, bo):
    from concourse.bass_utils import run_bass_kernel_spmd

    embeds = np.asarray(embeds, np.float32)
    Wq = np.asarray(Wq, np.float32)
    Wk = np.asarray(Wk, np.float32)
    Wv = np.asarray(Wv, np.float32)
    Wo = np.asarray(Wo, np.float32)
    bo = np.asarray(bo, np.float32)

    nc = _get_program()
    in_maps = make_in_maps(embeds, Wq, Wk, Wv, Wo)
    res = run_bass_kernel_spmd(nc, in_maps, list(range(N_CORES))).results
    out = np.empty((B, S, D), np.float32)
    for b in range(B):
        out[b] = res[2 * b]["o"] + res[2 * b + 1]["o"] + bo
    return out


# revision 14
# speedup vs baseline: 1.3397x; 1.0446x over previous
"""Trainium2 Bass kernel for causal multi-head attention with RoPE.

Problem: B=4, S=2048, D=768, H=12, HD=64 (torch-Linear style projections,
rotary embeddings on q/k, causal softmax, output projection + bias).

Sharding across 8 NeuronCores: core c handles batch c//2 and head-group
c%2 (6 of 12 heads). Each core computes a partial output projection
(its heads' contribution to ctx @ Wo.T); the host sums the two partials
per batch and adds the bias. No device collectives.

Per-core kernel (all matmul operands bf16, fp32 PSUM accumulation):
  - Q^T/K^T [hd, S] via pre-transposed weights. RoPE: rotate_half is a
    partition permutation done by 4 small PSUM->SBUF DMAs; then two
    full-partition DVE multiplies (cos, sign-folded sin) and a GpSimd add.
  - V [S, hd] per head with an appended ones column (row 64 of the PV
    accumulator becomes the softmax denominator for free).
  - Attention in 512-query windows, per head-pair: scores for heads A and
    B issue back-to-back with stationaries in PE row groups 0-63/64-127,
    so the two matmuls run concurrently (row tiling). One exp covers both
    heads' PSUM banks via a [128, 2, w] access pattern (scale=1/8 folded
    in, no max subtraction; scores are bounded). PV runs one k-chunk
    behind the exp stream so PE's in-order queue never stalls on ScalarE.
  - Denominator row is DMA-broadcast to 64 partitions, reciprocal'd with
    64 active lanes, and multiplied into the evicted ctx tile.
  - out = ctx^T-chunks.T @ Wo^T-chunks, interleaved with the tail of the
    attention so PE never idles; partial output summed on host.
"""

import numpy as np

B, S, D, H = 4, 2048, 768, 12
HD = D // H          # 64
N_CORES = 8
HEADS_PER_CORE = 6
PAIRS = 3            # head pairs per core
DC = D // 128        # 6 contraction chunks
NJ = S // 128        # 16 k-chunks
W = 512              # q-window width
NW = S // W          # 4 windows

_CACHE = {}


def _rope_tables():
    inv_freq = 1.0 / (10000.0 ** (np.arange(0, HD, 2, dtype=np.float64) / HD))
    ang = np.arange(S, dtype=np.float64)[:, None] * inv_freq[None, :]  # [S, 32]
    cos = np.cos(ang).astype(np.float32)   # [S, 32]
    sin = np.sin(ang).astype(np.float32)
    cosF = np.empty((128, S), np.float32)
    sinM = np.empty((128, S), np.float32)
    for g in range(4):
        cosF[32 * g:32 * g + 32] = cos.T
        sgn = -1.0 if g % 2 == 0 else 1.0
        sinM[32 * g:32 * g + 32] = sgn * sin.T
    return cosF, sinM


def _build_program(reps=1, dbg=False):
    import concourse.bacc as bacc
    import concourse.mybir as mybir
    import concourse.tile as tile

    f32 = mybir.dt.float32
    f32r = mybir.dt.float32r
    bf16 = mybir.dt.bfloat16
    AF = mybir.ActivationFunctionType
    OP = mybir.AluOpType

    nc = bacc.Bacc("TRN2", target_bir_lowering=False, debug=False,
                   num_devices=N_CORES)

    eT = nc.declare_dram_parameter("eT", [D, S], bf16, isOutput=False)
    wq = nc.declare_dram_parameter("wq", [D, 384], bf16, isOutput=False)
    wk = nc.declare_dram_parameter("wk", [D, 384], bf16, isOutput=False)
    wv = nc.declare_dram_parameter("wv", [D, 384], bf16, isOutput=False)
    wo = nc.declare_dram_parameter("wo", [384, D], bf16, isOutput=False)
    cosF_d = nc.declare_dram_parameter("cosF", [128, S], bf16, isOutput=False)
    sinM_d = nc.declare_dram_parameter("sinM", [128, S], bf16, isOutput=False)
    mask_d = nc.declare_dram_parameter("mask", [128, 128], bf16, isOutput=False)
    o = nc.declare_dram_parameter("o", [S, D], f32, isOutput=True)
    if dbg:
        qtd = nc.declare_dram_parameter("qtd", [128, PAIRS, S], bf16,
                                        isOutput=True)
        ktd = nc.declare_dram_parameter("ktd", [128, PAIRS, S], bf16,
                                        isOutput=True)
        vtd = nc.declare_dram_parameter("vtd", [128, NJ, HEADS_PER_CORE,
                                                HD + 1], bf16, isOutput=True)
        cxtd = nc.declare_dram_parameter("cxtd", [128, PAIRS, S], bf16,
                                         isOutput=True)

    with tile.TileContext(nc) as tc, \
            nc.allow_low_precision(reason="bf16 matmul operand tiles"):
        with tc.tile_pool(name="const", bufs=1) as cp:
            cosF = cp.tile([128, S], bf16)
            sinM = cp.tile([128, S], bf16)
            msk = cp.tile([128, 128], bf16)

            qt = cp.tile([128, PAIRS, S], bf16)
            kt = cp.tile([128, PAIRS, S], bf16)
            vt = cp.tile([128, NJ, HEADS_PER_CORE, HD + 1], f32r)
            nc.vector.memset(vt[:, :, :, HD].bitcast(mybir.dt.uint32),
                             0x3F800000)
            cxt = cp.tile([128, PAIRS, S], bf16)
            wot = cp.tile([128, PAIRS, D], bf16)

            eT_r = eT[:].rearrange("(n p) s -> p n s", p=128)

            for _rep in range(reps):
                with (
                    tc.tile_pool(name="asb", bufs=3) as asb,
                    tc.tile_pool(name="projsb", bufs=1) as pjs,
                ):
                    scp_cm = tc.tile_pool(name="scp", bufs=2, space="PSUM")
                    scp = scp_cm.__enter__()
                    cxp_cm = tc.tile_pool(name="cxp", bufs=2, space="PSUM")
                    cxp = cxp_cm.__enter__()
                    pps_cm = tc.tile_pool(name="pps", bufs=2, space="PSUM")
                    pps = pps_cm.__enter__()

                    def proj_chunk(pair, cc, with_v, wqt, wkt, wvt):
                        """One 512-col chunk of Q^T/K^T (+V when with_v)."""
                        cols = slice(512 * cc, 512 * cc + 512)
                        etA = pjs.tile([128, 3, 512], bf16, tag="et",
                                       bufs=3, name=f"eA{pair}{cc}")
                        nc.sync.dma_start(etA[:], eT_r[:, 0:3, cols])
                        etB = pjs.tile([128, 3, 512], bf16, tag="et",
                                       bufs=3, name=f"eB{pair}{cc}")
                        nc.sync.dma_start(etB[:], eT_r[:, 3:6, cols])

                        def et(d):
                            return (etA if d < 3 else etB)[:, d % 3, :]

                        for wt, dst in ((wqt, qt), (wkt, kt)):
                            ps = pps.tile([128, 512], f32, tag="ps",
                                          name=f"ps{pair}{cc}")
                            for d in range(DC):
                                nc.tensor.matmul(
                                    ps[:],
                                    wt[:, d, 128 * pair:128 * pair + 128],
                                    et(d),
                                    start=(d == 0), stop=(d == DC - 1))
                            # evict to bf16, then rotate_half as a
                            # partition-permuted SBUF->SBUF DMA
                            cpe = pjs.tile([128, 512], bf16, tag="cpe",
                                           bufs=2, name=f"e{pair}{cc}")
                            nc.vector.tensor_copy(cpe[:], ps[:])
                            cpr = pjs.tile([128, 512], bf16, tag="cpr",
                                           bufs=2, name=f"r{pair}{cc}")
                            for g in range(4):
                                gd = slice(32 * g, 32 * g + 32)
                                gs = slice(32 * (g ^ 1), 32 * (g ^ 1) + 32)
                                nc.scalar.dma_start(cpr[gd, :], cpe[gs, :])
                            t_t = pjs.tile([128, 512], bf16, tag="t",
                                           bufs=2, name=f"t{pair}{cc}")
                            nc.vector.tensor_tensor(
                                t_t[:], cpe[:], cosF[:, cols], OP.mult)
                            u_t = pjs.tile([128, 512], bf16, tag="u",
                                           bufs=2, name=f"u{pair}{cc}")
                            nc.vector.tensor_tensor(
                                u_t[:], cpr[:], sinM[:, cols], OP.mult)
                            nc.gpsimd.tensor_tensor(
                                dst[:, pair, cols], t_t[:], u_t[:], OP.add)
                        if with_v:
                            for i in range(4 * cc, 4 * cc + 4):
                                io = 128 * (i % 4)
                                pv = pps.tile([128, 384], f32, tag="ps",
                                              name=f"pv{cc}{i}")
                                for d in range(DC):
                                    nc.tensor.matmul(
                                        pv[:],
                                        et(d)[:, io:io + 128],
                                        wvt[:, d, :],
                                        start=(d == 0), stop=(d == DC - 1))
                                nc.vector.tensor_copy(vt[:, i, :, 0:HD], pv[:])

                    def attn_win(pair, w0):
                        """Attention for both heads of `pair` on q-window w0."""
                        base = W * w0
                        nj = 4 * w0 + 4
                        CA = cxp.tile([HD + 1, W], f32, tag="C",
                                      name=f"CA{pair}{w0}")
                        CB = cxp.tile([HD + 1, W], f32, tag="C",
                                      name=f"CB{pair}{w0}")

                        def emit_pv(j, et_, qlo):
                            off = qlo - base
                            wj = W - off
                            for hx, C in ((0, CA), (1, CB)):
                                nc.tensor.matmul(
                                    C[:, off:W],
                                    vt[:, j, 2 * pair + hx, :],
                                    et_[:, hx, 0:wj],
                                    start=(j == 0), stop=(j == nj - 1))

                        pend = None
                        for j in range(nj):
                            qlo = max(base, 128 * j)
                            wj = base + W - qlo
                            kk = slice(128 * j, 128 * j + 128)
                            sc = scp.tile([128, 2, W], f32, tag="sc",
                                          name=f"sc{pair}{w0}{j}")
                            nc.tensor.matmul(
                                sc[:, 0, 0:wj],
                                kt[0:HD, pair, kk],
                                qt[0:HD, pair, qlo:qlo + wj],
                                start=True, stop=True)
                            nc.tensor.matmul(
                                sc[:, 1, 0:wj],
                                kt[HD:128, pair, kk],
                                qt[HD:128, pair, qlo:qlo + wj],
                                start=True, stop=True)
                            et_ = asb.tile([128, 2, W], f32r, tag="ex",
                                           bufs=3, name=f"ex{pair}{w0}{j}")
                            nc.scalar.activation(
                                et_[:, :, 0:wj], sc[:, :, 0:wj], AF.Exp,
                                scale=0.125)
                            if qlo == 128 * j:   # diagonal: zero k > q
                                nc.gpsimd.tensor_tensor(
                                    et_[:, 0, 0:128], et_[:, 0, 0:128],
                                    msk[:], OP.mult)
                                nc.gpsimd.tensor_tensor(
                                    et_[:, 1, 0:128], et_[:, 1, 0:128],
                                    msk[:], OP.mult)
                            # software pipeline: PV runs one j behind so
                            # PE's in-order queue never waits on exp_j
                            if pend is not None:
                                emit_pv(*pend)
                            pend = (j, et_, qlo)
                        emit_pv(*pend)

                        cs = slice(base, base + W)
                        rec = asb.tile([128, W], bf16, tag="rec", bufs=2,
                                       name=f"rc{pair}{w0}")
                        for hx, C in ((0, CA), (1, CB)):
                            po = HD * hx
                            nc.vector.tensor_copy(cxt[po:po + HD, pair, cs],
                                                  C[0:HD, :])
                            rr = asb.tile([1, W], bf16, tag="rr", bufs=2,
                                          name=f"rr{pair}{w0}{hx}")
                            nc.vector.reciprocal(rr[:], C[HD:HD + 1, :])
                            nc.sync.dma_start(
                                rec[po:po + HD, :],
                                rr[0:1, None, :].to_broadcast([1, HD, W]))
                            nc.vector.tensor_tensor(
                                cxt[po:po + HD, pair, cs],
                                cxt[po:po + HD, pair, cs],
                                rec[po:po + HD, :], OP.mult)

                    def out_chunk(i, osp):
                        op_ = osp.tile([128, D], f32, tag="op", name=f"op{i}")
                        ss = slice(128 * i, 128 * i + 128)
                        for pair in range(PAIRS):
                            for c0 in range(0, D, 512):
                                cw = min(512, D - c0)
                                nc.tensor.matmul(
                                    op_[:, c0:c0 + cw],
                                    cxt[:, pair, ss],
                                    wot[:, pair, c0:c0 + cw],
                                    start=(pair == 0),
                                    stop=(pair == PAIRS - 1))
                        ot = asb.tile([128, D], f32, tag="ot", bufs=3,
                                      name=f"ot{i}")
                        nc.vector.tensor_copy(ot[:], op_[:])
                        eng = nc.sync if i % 2 == 0 else nc.scalar
                        eng.dma_start(o[ss, :], ot[:])

                    # weights/tables on the ScalarE DMA queue so the eT
                    # stream (SP queue) starts immediately
                    wqt = pjs.tile([128, DC, 384], bf16)
                    nc.scalar.dma_start(
                        wqt[:], wq[:].rearrange("(n p) m -> p n m", p=128))
                    wkt = pjs.tile([128, DC, 384], bf16)
                    nc.scalar.dma_start(
                        wkt[:], wk[:].rearrange("(n p) m -> p n m", p=128))
                    wvt = pjs.tile([128, DC, 384], bf16)
                    nc.scalar.dma_start(
                        wvt[:], wv[:].rearrange("(n p) m -> p n m", p=128))
                    nc.scalar.dma_start(cosF[:], cosF_d[:])
                    nc.scalar.dma_start(sinM[:], sinM_d[:])
                    nc.scalar.dma_start(msk[:], mask_d[:])
                    nc.sync.dma_start(
                        wot[:], wo[:].rearrange("(n p) m -> p n m", p=128))

                    # pipeline: V + pair p projections feed attention windows;
                    # next pair's projections overlap current pair's attention
                    proj_chunk(0, 0, True, wqt, wkt, wvt)
                    proj_chunk(0, 1, True, wqt, wkt, wvt)
                    attn_win(0, 0)
                    proj_chunk(0, 2, True, wqt, wkt, wvt)
                    attn_win(0, 1)
                    proj_chunk(0, 3, True, wqt, wkt, wvt)
                    attn_win(0, 2)
                    proj_chunk(1, 0, False, wqt, wkt, wvt)
                    attn_win(0, 3)
                    proj_chunk(1, 1, False, wqt, wkt, wvt)
                    attn_win(1, 0)
                    proj_chunk(1, 2, False, wqt, wkt, wvt)
                    attn_win(1, 1)
                    proj_chunk(1, 3, False, wqt, wkt, wvt)
                    attn_win(1, 2)
                    proj_chunk(2, 0, False, wqt, wkt, wvt)
                    attn_win(1, 3)
                    proj_chunk(2, 1, False, wqt, wkt, wvt)
                    proj_chunk(2, 2, False, wqt, wkt, wvt)
                    proj_chunk(2, 3, False, wqt, wkt, wvt)
                    # projection PSUM banks are dead now — recycle for the
                    # output projection so it overlaps pair-2 attention
                    pps_cm.__exit__(None, None, None)
                    osp_cm = tc.tile_pool(name="osp", bufs=1, space="PSUM")
                    osp = osp_cm.__enter__()
                    attn_win(2, 0)
                    attn_win(2, 1)
                    for i in range(0, 4):
                        out_chunk(i, osp)
                    attn_win(2, 2)
                    for i in range(4, 8):
                        out_chunk(i, osp)
                    attn_win(2, 3)
                    for i in range(8, 12):
                        out_chunk(i, osp)
                    osp_cm.__exit__(None, None, None)
                    cxp_cm.__exit__(None, None, None)
                    scp_cm.__exit__(None, None, None)
                    osp2_cm = tc.tile_pool(name="osp2", bufs=2, space="PSUM")
                    osp2 = osp2_cm.__enter__()
                    for i in range(12, 16):
                        out_chunk(i, osp2)
                    osp2_cm.__exit__(None, None, None)
                    if dbg:
                        nc.sync.dma_start(qtd[:], qt[:])
                        nc.sync.dma_start(ktd[:], kt[:])
                        nc.sync.dma_start(vtd[:], vt[:])
                        nc.sync.dma_start(cxtd[:], cxt[:])

    nc.compile()
    return nc


def _get_program(reps=1):
    if reps not in _CACHE:
        _CACHE[reps] = _build_program(reps)
    return _CACHE[reps]


def make_in_maps(embeds, Wq, Wk, Wv, Wo):
    import ml_dtypes
    bf16 = ml_dtypes.bfloat16
    cosF, sinM = _rope_tables()
    cosF, sinM = cosF.astype(bf16), sinM.astype(bf16)
    mask = (np.arange(128)[:, None] <= np.arange(128)[None, :]).astype(bf16)
    eTs = [np.ascontiguousarray(embeds[b].T).astype(bf16) for b in range(B)]
    in_maps = []
    for c in range(N_CORES):
        b, hg = c // 2, c % 2
        hs = slice(hg * 384, hg * 384 + 384)
        in_maps.append({
            "eT": eTs[b],
            "wq": np.ascontiguousarray(Wq[hs].T).astype(bf16),
            "wk": np.ascontiguousarray(Wk[hs].T).astype(bf16),
            "wv": np.ascontiguousarray(Wv[hs].T).astype(bf16),
            "wo": np.ascontiguousarray(Wo[:, hs].T).astype(bf16),
            "cosF": cosF, "sinM": sinM, "mask": mask,
        })
    return in_maps


def kernel(embeds, Wq, Wk, Wv, Wo, bo):
    from concourse.bass_utils import run_bass_kernel_spmd

    embeds = np.asarray(embeds, np.float32)
    Wq = np.asarray(Wq, np.float32)
    Wk = np.asarray(Wk, np.float32)
    Wv = np.asarray(Wv, np.float32)
    Wo = np.asarray(Wo, np.float32)
    bo = np.asarray(bo, np.float32)

    nc = _get_program()
    in_maps = make_in_maps(embeds, Wq, Wk, Wv, Wo)
    res = run_bass_kernel_spmd(nc, in_maps, list(range(N_CORES))).results
    out = np.empty((B, S, D), np.float32)
    for b in range(B):
        out[b] = res[2 * b]["o"] + res[2 * b + 1]["o"] + bo
    return out
